# revision 54
# baseline (speedup 1.0000x reference)
"""Coattention model kernel for 8 Trainium2 NeuronCores.

Data-parallel over batch (B=16 -> 2 samples/core).  Per sample:
  e_corr = w_e @ e, q_corr = w_q @ q           (1x1 convs, bf16 matmuls)
  A[n,m] = e_corr[:,n] . q_corr[:,m]           (PSUM f32)
  E_B = exp(A),  rB = 1/rowsum  (accum_out)    (softmax over m)
  E_A = exp(A^T), rA = 1/rowsum (recompute)    (softmax over n)
  exemplar_att = (q^T * rB)^T-matmul E_B       (scale folded into qT)
  query_att    = (e^T * rA)^T-matmul E_A
  out[0:512]   = conv3x3(exemplar_att, w_c1)   (Winograd F(4,3) vertical)
  out[512:1024]= conv3x3(query_att,    w_c2)

The 3x3 convs use 1-D Winograd F(4,3) along the vertical axis: per
group of 4 output rows the vertical taps collapse from 12 multiplies
to 6 (6 nu-terms shared by 4 rows), halving conv PE work.  The input
transform (6 combo images) is split Vector/GpSimd, the output
transform (A^T rows over M0..M5) runs Vector/GpSimd with two M's
staged through SBUF (TensorTensor reads at most one PSUM operand),
and the weight transform happens on the host.

Host pre-transposes/casts all operands so the device does no layout work.
"""

import os
import sys
import types
import numpy as np
import ml_dtypes

import concourse.bass as bass
import concourse.mybir as mybir
import concourse.tile as tile
from concourse import bacc
from concourse.bass_utils import run_bass_kernel_spmd

BF16 = mybir.dt.bfloat16
F32 = mybir.dt.float32
AX = mybir.AxisListType.X
EXP = mybir.ActivationFunctionType.Exp
COPY = mybir.ActivationFunctionType.Copy

B, C, HH, WW = 16, 512, 40, 40
N = HH * WW                      # 1600
NCORES = 8
BS = B // NCORES                 # 2 samples per core
CP = 256                         # correlation dim
CC = C // 128                    # 4 channel chunks
PC = CP // 128                   # 2 correlation chunks
NB = (N + 127) // 128            # 13 row blocks (12x128 + 64)
NCH = [(0, 512), (512, 512), (1024, 512), (1536, 64)]   # free-dim chunks of N
YCH = [(0, 12), (12, 12), (24, 12), (36, 4)]            # att row chunks
ACH = [(0, 512), (512, 512), (1024, 512), (1536, 64)]   # A-row chunks
PW = WW + 2                      # 42
PH = 44                          # padded rows 0..41 + 2 alignment rows
NPAD = PH * PW                   # 1848
NT = 10                          # winograd tile rows (4 outputs each)


def _rows(b):
    return 128 if b < NB - 1 else N - 128 * (NB - 1)


_CACHED = None


def _build_program():
    # COATT_LIMIT: debug stage cutoff ("proj", "apass", "att", "" = full)
    limit = os.environ.get("COATT_LIMIT", "")
    nbs = int(os.environ.get("COATT_BS", str(BS)))
    nc = bacc.Bacc("TRN2", target_bir_lowering=False, debug=False,
                   num_devices=NCORES)

    e_d = nc.dram_tensor("e", [BS, C, N], BF16, kind="ExternalInput")
    q_d = nc.dram_tensor("q", [BS, C, N], BF16, kind="ExternalInput")
    et_d = nc.dram_tensor("et", [BS, N, C], BF16, kind="ExternalInput")
    qt_d = nc.dram_tensor("qt", [BS, N, C], BF16, kind="ExternalInput")
    wet_d = nc.dram_tensor("wet", [C, CP], BF16, kind="ExternalInput")
    wqt_d = nc.dram_tensor("wqt", [C, CP], BF16, kind="ExternalInput")
    # Winograd-transformed conv weights: [occ, ic, (nu*3+dx)*128 + oc_w]
    u1_d = nc.dram_tensor("u1", [CC, C, 18 * 128], BF16, kind="ExternalInput")
    u2_d = nc.dram_tensor("u2", [CC, C, 18 * 128], BF16, kind="ExternalInput")
    out_d = nc.dram_tensor("out", [BS, 2 * C, N], F32, kind="ExternalOutput")

    with tile.TileContext(nc) as tc:
        with (
            tc.tile_pool(name="wproj", bufs=2) as wproj_p,
            tc.tile_pool(name="scratch", bufs=8) as scratch_p,
            tc.tile_pool(name="eqt", bufs=2) as eqt_p,
            tc.tile_pool(name="corr", bufs=2) as corr_p,
            tc.tile_pool(name="big", bufs=1) as big_p,
            tc.tile_pool(name="u", bufs=2) as u_p,
            tc.tile_pool(name="v",
                         bufs=int(os.environ.get("COATT_VBUFS", "7"))) as v_p,
            tc.tile_pool(name="st", bufs=2) as st_p,
            tc.tile_pool(name="tmp", bufs=7) as tmp_p,
            tc.tile_pool(name="stats", bufs=8) as stats_p,
            tc.tile_pool(name="ostage", bufs=2) as ostage_p,
            tc.tile_pool(name="ps", bufs=4, space=bass.MemorySpace.PSUM) as ps_p,
        ):
            # --- projection weights, loaded once (e-weights first so the
            # first projection chain can start after just 8 DMAs) ---------
            wet_t = wproj_p.tile([128, CC * CP], BF16, tag="wproj")
            wqt_t = wproj_p.tile([128, CC * CP], BF16, tag="wproj")
            for cc in range(CC):
                nc.sync.dma_start(wet_t[:, cc * CP:(cc + 1) * CP],
                                  wet_d[cc * 128:(cc + 1) * 128, :])

            def _dbg_out(s, tiles_bf16):
                """Debug-mode: write bf16 tiles into out so stages stay live."""
                for idx, t in enumerate(tiles_bf16):
                    st = ostage_p.tile([128, 512], F32, tag="ostage")
                    nc.vector.tensor_copy(st[:, :], t[:, :512])
                    nc.sync.dma_start(out_d[s, (idx % 8) * 128:(idx % 8 + 1) * 128,
                                            :512], st[:, :])

            for s in range(nbs):
                # --- input loads ----------------------------------------
                e_t = []
                q_t = []
                for cc in range(CC):
                    t = scratch_p.tile([128, N], BF16, tag="scratch",
                                       padded_shape=[128, NPAD])
                    nc.sync.dma_start(t[:, :], e_d[s, cc * 128:(cc + 1) * 128, :])
                    e_t.append(t)
                if s == 0:
                    for cc in range(CC):
                        nc.sync.dma_start(wqt_t[:, cc * CP:(cc + 1) * CP],
                                          wqt_d[cc * 128:(cc + 1) * 128, :])
                for cc in range(CC):
                    t = scratch_p.tile([128, N], BF16, tag="scratch",
                                       padded_shape=[128, NPAD])
                    nc.sync.dma_start(t[:, :], q_d[s, cc * 128:(cc + 1) * 128, :])
                    q_t.append(t)
                # --- 1x1 projections ------------------------------------
                # +64 tail cols so block-12 stationary slices can read a
                # full 128 rows (values there are never consumed: the
                # resulting PSUM rows 64.. are not exp'd or copied out).
                ecorr_t = corr_p.tile([128, PC * N + 64], BF16, tag="corr")
                qcorr_t = corr_p.tile([128, PC * N + 64], BF16, tag="corr")
                for (w_t, x_t, o_t) in ((wet_t, e_t, ecorr_t),
                                        (wqt_t, q_t, qcorr_t)):
                    for oc in range(PC):
                        for j, (n0, nw) in enumerate(NCH):
                            if j % 2 == 0:
                                pair = ps_p.tile([128, 1024], F32, tag="ps")
                            ps = pair[:, (j % 2) * 512:(j % 2) * 512 + nw]
                            for cc in range(CC):
                                nc.tensor.matmul(
                                    ps,
                                    w_t[:, cc * CP + oc * 128: cc * CP + (oc + 1) * 128],
                                    x_t[cc][:, n0:n0 + nw],
                                    start=(cc == 0), stop=(cc == CC - 1))
                            nc.vector.tensor_copy(
                                o_t[:, oc * N + n0: oc * N + n0 + nw], ps)

                # Zero the 64 pad rows of block 12 so full-128-row att
                # matmuls contract zeros there (0 * 0-padded E = 0).
                et_t = eqt_p.tile([128, NB * C], BF16, tag="eqt")
                qt_t = eqt_p.tile([128, NB * C], BF16, tag="eqt")
                rl = _rows(NB - 1)
                nc.gpsimd.memset(et_t[rl:, (NB - 1) * C:NB * C], 0.0)
                nc.gpsimd.memset(qt_t[rl:, (NB - 1) * C:NB * C], 0.0)
                for b in range(NB):
                    r = _rows(b)
                    nc.sync.dma_start(et_t[:r, b * C:(b + 1) * C],
                                      et_d[s, b * 128:b * 128 + r, :])
                    nc.sync.dma_start(qt_t[:r, b * C:(b + 1) * C],
                                      qt_d[s, b * 128:b * 128 + r, :])

                # --- A-pass: matmul + unnormalized exp ------------------
                # A values are O(1) (inputs are N(0,1), weights 0.01*N(0,1)),
                # so exp without max-subtraction is safe; normalization is
                # folded into the transposed operand's rows (1/rowsum),
                # with the row sums coming free from exp's accum_out.
                AB = [(0, ((0, 512), (512, 512)), 1024),
                      (1024, ((0, 512), (512, 64)), 576)]

                def a_pass(lcorr, rcorr, E_t, tgt_t):
                    for b in range(NB):
                        r = _rows(b)
                        for base, chunks, wt in AB:
                            ps = ps_p.tile([128, 1024], F32, tag="ps")
                            for (off, nw) in chunks:
                                for cc in range(PC):
                                    # always 128 stationary rows: block 12
                                    # pads with junk whose PSUM rows are
                                    # never exp'd (see corr tail comment)
                                    nc.tensor.matmul(
                                        ps[:, off:off + nw],
                                        lcorr[:, cc * N + b * 128:
                                              cc * N + b * 128 + 128],
                                        rcorr[:, cc * N + base + off:
                                              cc * N + base + off + nw],
                                        start=(cc == 0), stop=(cc == PC - 1))
                            nc.scalar.activation(
                                E_t[:r, b * N + base: b * N + base + wt],
                                ps[:r, :wt], EXP)
                        rs = stats_p.tile([128, 1], F32, tag="rs")
                        nc.vector.reduce_sum(rs[:r, :],
                                             E_t[:r, b * N:(b + 1) * N], axis=AX)
                        rc = stats_p.tile([128, 1], F32, tag="rc")
                        nc.vector.reciprocal(rc[:r, :], rs[:r, :])
                        nc.vector.tensor_scalar_mul(
                            tgt_t[:r, b * C:(b + 1) * C],
                            tgt_t[:r, b * C:(b + 1) * C], rc[:r, :])

                # --- attention matmul into padded images ----------------
                # (pad copies on the Scalar engine; Vector does winograd)
                def att(tgtT_t, E_t, pads):
                    half = 0
                    for oc in range(CC):
                        pad3 = pads[oc].rearrange("p (a b) -> p a b", a=PH)
                        for (y0, ny) in YCH:
                            nw = ny * WW
                            if half % 2 == 0:
                                pair = ps_p.tile([128, 1024], F32, tag="ps")
                            ps = pair[:, (half % 2) * 512:(half % 2) * 512 + 512]
                            half += 1
                            for b in range(NB):
                                nc.tensor.matmul(
                                    ps[:, :nw],
                                    tgtT_t[:, b * C + oc * 128: b * C + (oc + 1) * 128],
                                    E_t[:, b * N + y0 * WW: b * N + y0 * WW + nw],
                                    start=(b == 0), stop=(b == NB - 1))
                            nc.scalar.activation(
                                pad3[:, 1 + y0:1 + y0 + ny, 1:1 + WW],
                                ps[:, :nw].rearrange("p (a b) -> p a b", a=ny),
                                COPY)

                # --- winograd F(4,3) input transform ----------------------
                # tiles of 6 pad rows 4i+k (k=0..5), i=0..9; B^T rows:
                # V0=4d0-5d2+d4  V1=-4(d1+d2)+d3+d4  V2=4(d1-d2)-d3+d4
                # V3=2(d3-d1)+d4-d2  V4=-2(d3-d1)+d4-d2  V5=4d1-5d3+d5
                MUL = mybir.AluOpType.mult
                ADD = mybir.AluOpType.add
                SUB = mybir.AluOpType.subtract

                def vtrans_one(pad, v, fast):
                    pad5 = pad.rearrange("p (i k x) -> p i k x",
                                         i=PH // 4, k=4)
                    d = [pad5[:, k // 4: k // 4 + NT, k % 4, :]
                         for k in range(6)]
                    eng2 = nc.vector if fast else nc.gpsimd

                    def tmp():
                        return tmp_p.tile([128, NT, PW], F32, tag="tmp",
                                          name="vtmp")

                    # GpSimd (Pool) supports only plain TensorTensor on
                    # SBUF: it takes the add/sub combos, Vector the STTs.
                    t0 = tmp()
                    nc.vector.scalar_tensor_tensor(t0, d[2], -5.0, d[4], MUL, ADD)
                    nc.vector.scalar_tensor_tensor(v[:, 0, :, :], d[0], 4.0,
                                                   t0, MUL, ADD)
                    a1, b1 = tmp(), tmp()
                    eng2.tensor_add(a1, d[1], d[2])
                    eng2.tensor_add(b1, d[3], d[4])
                    nc.vector.scalar_tensor_tensor(v[:, 1, :, :], a1, -4.0,
                                                   b1, MUL, ADD)
                    a2, b2 = tmp(), tmp()
                    eng2.tensor_sub(a2, d[1], d[2])
                    eng2.tensor_sub(b2, d[4], d[3])
                    nc.vector.scalar_tensor_tensor(v[:, 2, :, :], a2, 4.0,
                                                   b2, MUL, ADD)
                    cx, ex = tmp(), tmp()
                    eng2.tensor_sub(cx, d[3], d[1])
                    eng2.tensor_sub(ex, d[4], d[2])
                    nc.vector.scalar_tensor_tensor(v[:, 3, :, :], cx, 2.0,
                                                   ex, MUL, ADD)
                    nc.vector.scalar_tensor_tensor(v[:, 4, :, :], cx, -2.0,
                                                   ex, MUL, ADD)
                    t5 = tmp()
                    nc.vector.scalar_tensor_tensor(t5, d[3], -5.0, d[5], MUL, ADD)
                    nc.vector.scalar_tensor_tensor(v[:, 5, :, :], d[1], 4.0,
                                                   t5, MUL, ADD)

                def vtrans(pads, ccs=range(CC), vt=None, fast=False):
                    vt = [None] * CC if vt is None else vt
                    for ccx in ccs:
                        v = v_p.tile([128, 6, NT, PW], BF16, tag="v")
                        vtrans_one(pads[ccx], v, fast)
                        vt[ccx] = v
                    return vt

                # --- winograd F(4,3) 3x3 conv ----------------------------
                # M_nu accumulated over (dx, icc); A^T output rows:
                # Y0=M0+M1+M2+M3+M4  Y1=(M1-M2)+2(M3-M4)
                # Y2=(M1+M2)+4(M3+M4)  Y3=(M1-M2)+8(M3-M4)+M5
                # M2/M4 staged through SBUF so every TT op reads <=1 PSUM.
                def conv_wino(u_d, vt, out_base, split_last=False):
                    for occ in range(CC):
                        u = u_p.tile([128, CC * 18 * 128], BF16, tag="u")
                        for icc in range(CC):
                            nc.sync.dma_start(
                                u[:, icc * 2304:(icc + 1) * 2304],
                                u_d[occ, icc * 128:(icc + 1) * 128, :])
                        chunks = ([(0, 5), (5, 5)]
                                  if split_last and occ == CC - 1
                                  else [(0, NT)])
                        for (i0, nt) in chunks:
                            nwc = nt * WW
                            Ms = []
                            for nu in range(6):
                                if nu % 2 == 0:
                                    pair = ps_p.tile([128, 1024], F32,
                                                     tag="ps")
                                ps = pair[:, (nu % 2) * 512:
                                          (nu % 2) * 512 + nwc]
                                idx = 0
                                for dx in range(3):
                                    for icc in range(CC):
                                        o = icc * 2304 + (nu * 3 + dx) * 128
                                        nc.tensor.matmul(
                                            ps.rearrange("p (a b) -> p a b",
                                                         a=nt),
                                            u[:, o:o + 128],
                                            vt[icc][:, nu, i0:i0 + nt,
                                                    dx:dx + WW],
                                            start=(idx == 0), stop=(idx == 11))
                                        idx += 1
                                Ms.append(ps)
                            st = st_p.tile([128, NT * 4 * WW], F32, tag="st",
                                           name="st")
                            stv = st[:, :nt * 4 * WW].rearrange(
                                "p (a q b) -> p a q b", a=nt, q=4)

                            def ytmp():
                                return tmp_p.tile([128, nwc], F32, tag="tmp",
                                                  name="ytmp",
                                                  padded_shape=[128, NT * PW])
                            m2s, m4s = ytmp(), ytmp()
                            nc.scalar.activation(m2s, Ms[2], COPY)
                            nc.scalar.activation(m4s, Ms[4], COPY)
                            # PSUM-reading ops go on Vector (GpSimd cannot
                            # touch PSUM); SBUF-only combos go on GpSimd.
                            ax, gx, hx, ix = ytmp(), ytmp(), ytmp(), ytmp()
                            nc.vector.tensor_add(ax, m4s, Ms[3])   # M3+M4
                            nc.vector.tensor_sub(gx, Ms[3], m4s)   # M3-M4
                            nc.vector.tensor_sub(hx, Ms[1], m2s)   # M1-M2
                            nc.vector.tensor_add(ix, m2s, Ms[1])   # M1+M2
                            # PSUM-reading ops first so the M pairs free up
                            # for the next occ's chains; SBUF-only combos
                            # follow.
                            ty0, ty3 = ytmp(), ytmp()
                            nc.vector.tensor_add(ty0, ix, Ms[0])
                            nc.vector.scalar_tensor_tensor(ty3, gx, 8.0, hx,
                                                           MUL, ADD)
                            nc.vector.tensor_add(
                                stv[:, :, 3, :],
                                ty3.rearrange("p (a b) -> p a b", a=nt),
                                Ms[5].rearrange("p (a b) -> p a b", a=nt))
                            yeng = nc.vector if split_last and occ == CC - 1 \
                                else nc.gpsimd
                            yeng.tensor_add(
                                stv[:, :, 0, :],
                                ty0.rearrange("p (a b) -> p a b", a=nt),
                                ax.rearrange("p (a b) -> p a b", a=nt))
                            nc.vector.scalar_tensor_tensor(
                                stv[:, :, 1, :],
                                gx.rearrange("p (a b) -> p a b", a=nt), 2.0,
                                hx.rearrange("p (a b) -> p a b", a=nt),
                                MUL, ADD)
                            nc.vector.scalar_tensor_tensor(
                                stv[:, :, 2, :],
                                ax.rearrange("p (a b) -> p a b", a=nt), 4.0,
                                ix.rearrange("p (a b) -> p a b", a=nt),
                                MUL, ADD)
                            nc.sync.dma_start(
                                out_d[s, out_base + occ * 128:
                                      out_base + (occ + 1) * 128,
                                      4 * i0 * WW: (4 * i0 + 4 * nt) * WW],
                                st[:, :nt * 4 * WW])

                # E = exp(A); E^T built by a second matmul pass (recompute
                # keeps only one E buffer live -> SBUF for winograd tiles).
                if limit == "proj":
                    _dbg_out(s, [ecorr_t, qcorr_t])
                    continue
                EB_t = big_p.tile([128, NB * N], BF16, tag="big")
                nc.gpsimd.memset(EB_t[rl:, (NB - 1) * N:NB * N], 0.0)
                a_pass(ecorr_t, qcorr_t, EB_t, qt_t)
                if limit == "apass":
                    _dbg_out(s, [EB_t, qt_t])
                    continue
                pads_e = []
                for oc in range(CC):
                    t = scratch_p.tile([128, NPAD], BF16, tag="scratch")
                    nc.gpsimd.memset(t[:, :], 0.0)
                    pads_e.append(t)
                att(qt_t, EB_t, pads_e)

                EA_t = big_p.tile([128, NB * N], BF16, tag="big")
                nc.gpsimd.memset(EA_t[rl:, (NB - 1) * N:NB * N], 0.0)
                a_pass(qcorr_t, ecorr_t, EA_t, et_t)
                # vt_e emitted after apass2 so its Vector ops don't delay
                # apass2's rowsum reduces; it runs during att2 on the PE.
                vt_e = vtrans(pads_e)
                pads_q = []
                for oc in range(CC):
                    t = scratch_p.tile([128, NPAD], BF16, tag="scratch")
                    nc.gpsimd.memset(t[:, :], 0.0)
                    pads_q.append(t)
                att(et_t, EA_t, pads_q)

                if limit == "att":
                    _dbg_out(s, pads_e + pads_q)
                    continue
                # vt_q tiles with free v-slots go before conv1; the rest
                # must wait for conv1 to release vt_e slots, so their ops
                # are emitted after conv1 (emitting them earlier would
                # head-of-line-block conv1's Y-transform drains) and run
                # on Vector only so conv2's first chains start quickly.
                nvq = int(os.environ.get("COATT_VBUFS", "7")) - 4
                vt_q = vtrans(pads_q, range(min(nvq, CC)))
                conv_wino(u1_d, vt_e, 0)
                vtrans(pads_q, range(min(nvq, CC), CC), vt_q, fast=True)
                conv_wino(u2_d, vt_q, C, split_last=(s == nbs - 1))

    nc.compile()
    return nc


def _get_program():
    global _CACHED
    if _CACHED is None:
        _CACHED = _build_program()
    return _CACHED


def _install_ntff_hook():
    """Register the axon NTFF profiling hook if the shim module is absent."""
    if "antenv.axon_hooks" in sys.modules:
        return
    try:
        import antenv
        from trn_agent_boot.trn_boot import _ntff_profile_via_ctypes
    except ImportError:
        return
    mod = types.ModuleType("antenv.axon_hooks")
    _h = [None]
    mod.get_axon_ntff_profile_hook = lambda: _h[0]
    mod.set_axon_ntff_profile_hook = lambda v: _h.__setitem__(0, v)
    sys.modules["antenv.axon_hooks"] = mod
    antenv.axon_hooks = mod
    so = "/opt/axon/libaxon_pjrt.so"
    if os.path.exists(so):
        mod.set_axon_ntff_profile_hook(_ntff_profile_via_ctypes(so))


LAST_RESULTS = None  # BassKernelResults of the most recent run (for test.py)


_G43 = np.array([[1 / 4, 0, 0],
                 [-1 / 6, -1 / 6, -1 / 6],
                 [-1 / 6, 1 / 6, -1 / 6],
                 [1 / 24, 1 / 12, 1 / 6],
                 [1 / 24, -1 / 12, 1 / 6],
                 [0, 0, 1]], dtype=np.float32)


def _wino_u(w):
    """[O,I,3,3] conv weights -> winograd-F(4,3)-vertical [occ, ic, 18*128]."""
    bf = ml_dtypes.bfloat16
    w = np.asarray(w, dtype=np.float32)
    u = np.einsum('nk,oikx->oinx', _G43, w)         # [O, I, nu, dx]
    u = u.reshape(CC, 128, C, 18)                   # [occ, oc_w, ic, nu*3+dx]
    u = u.transpose(0, 2, 3, 1)                     # [occ, ic, nu*3+dx, oc_w]
    return np.ascontiguousarray(u.reshape(CC, C, 18 * 128).astype(bf))


def prep_in_maps(exemplar, query, w_e, w_q, w_c1, w_c2):
    bf = ml_dtypes.bfloat16
    ex = np.asarray(exemplar, dtype=np.float32).reshape(B, C, N)
    qu = np.asarray(query, dtype=np.float32).reshape(B, C, N)
    e_b = ex.astype(bf)
    q_b = qu.astype(bf)
    et_b = np.ascontiguousarray(e_b.transpose(0, 2, 1))
    qt_b = np.ascontiguousarray(q_b.transpose(0, 2, 1))
    wet = np.ascontiguousarray(
        np.asarray(w_e, dtype=np.float32).reshape(CP, C).T.astype(bf))
    wqt = np.ascontiguousarray(
        np.asarray(w_q, dtype=np.float32).reshape(CP, C).T.astype(bf))
    u1 = _wino_u(w_c1)
    u2 = _wino_u(w_c2)

    in_maps = []
    for k in range(NCORES):
        s0 = k * BS
        in_maps.append({
            "e": e_b[s0:s0 + BS], "q": q_b[s0:s0 + BS],
            "et": et_b[s0:s0 + BS], "qt": qt_b[s0:s0 + BS],
            "wet": wet, "wqt": wqt, "u1": u1, "u2": u2,
        })
    return in_maps


def kernel(exemplar, query, w_e, w_q, w_c1, w_c2):
    in_maps = prep_in_maps(exemplar, query, w_e, w_q, w_c1, w_c2)
    nc = _get_program()
    res = run_bass_kernel_spmd(nc, in_maps, core_ids=list(range(NCORES)),
                               trace=False)
    global LAST_RESULTS
    LAST_RESULTS = res
    out = np.concatenate([res.results[k]["out"] for k in range(NCORES)], axis=0)
    return np.ascontiguousarray(out.reshape(B, 2 * C, HH, WW))
